# revision 43
# baseline (speedup 1.0000x reference)
"""Trainium2 kernel for nn_Decoder_featurizer: HRR decoder featurization.

reference: out = concat([p, l, assoc(dec_d, p)..., assoc(dec_d, l)...], -1)
where assoc(d, x)[j] = sum_t d[t] * x[(j+t) % N]  (circular correlation).

Circular correlation is a circulant matmul y = x @ C_d with
C_d[k, j] = dec[d, (k-j) % N].  v5 cuts the PE work 3.5x below the dense
circulant by composing two matrix identities, recursively:

 (1) CRT split of a cyclic ring:  x^2M-1 = (x^M-1)(x^M+1) turns a
     circulant-2M matmul into a circulant-M plus a negacyclic-M matmul on
     folded inputs (2 half-size mults instead of 4 quarters).
 (2) Karatsuba on the negacyclic ring: the skew-circulant has block form
     [[P, -R], [R, P]] (complex-multiplication structure), so it costs 3
     half-size Toeplitz matmuls (m1=x0@P, m2=x1@R, m3=(x0+x1)@(P-R)) with
     the butterfly  y = [m1+m2, m3-m1+m2].

Applied to N=1024:  cyc1024 -> cyc512 + neg512;  cyc512 -> cyc256 + neg256;
cyc256 -> cyc128 + neg128;  neg512 -> 3 x Toep256 (Karatsuba).  Leaves per
decoder:
  m1,m2,m3     : 3 matmuls [B,256]@[256,256]   (neg512 Karatsuba)
  Qm_lo,Qm_hi  : 4 matmuls [B,128]@[128,128]   (neg256 direct block form,
                 pairs PSUM-accumulated: Qm_lo = x0@P2 + x1@R2,
                 Qm_hi = x1@P2 - x0@R2 -- trades 1 extra small matmul for
                 shipping 256 instead of 384 partial cols; the drain/DMA
                 path, not the PE, is the binding budget)
  q            : [B,128]@circ128               (cyc128)
  s            : [B,128]@skew128               (neg128)
= 245760 MACs/row/decoder vs 1048576 dense.  All folded x operands are
host-precomputed (shipped transposed, bf16); the remaining butterfly
recombines run on the host in f32.  The passthrough columns never touch
the device.

Every device matmul has N=512: weights are stored decoder-contiguous so one
moving operand covers 2 decoders (256-wide leaves) or 4 decoders (128-wide
leaves) -- small-N matmuls measurably pay a large per-MM floor (v4 post-
mortem).  v7 organizes the work as single-leaf PSUM phases spanning 8
decoders (256-wide leaves, 4 pairs) or all 16 (128-wide leaves), so each
stationary LDWEIGHTS feeds 4 N=512 matmuls.  Per m-tile: 10 phases of 4
PSUM banks rotating through 2 pool buffers; DVE and ScalarE split each
phase's psum->SBUF bf16 drain so banks free within the next phase's PE
window; all drains land in one stage tile and a single 5.2 MB DMA per
m-tile writes out at ~97%% DMA efficiency.  Device output is raw partials
in bf16, phase-major.  Redundant LDWEIGHTS are deduped at the BIR level.
"""

import numpy as np
import ml_dtypes

HRR = 1024
D = 16
B = 8192
NCORES = 8
BPC = B // NCORES            # batch rows per core
ROWS = 2 * BPC               # rows per core (problem + lemma stacked)
DN = D * HRR                 # 16384 assoc features per input
OUT_COLS = 2 * HRR + 2 * DN  # 34816
DEV_COLS = 20480             # device partial columns per row
FLUSH_COLS = 4096            # stage-tile flush granularity (1 MB DMAs)
# phase-major column layout: m1,m2,m3 [16 dec x 256], Qm_lo,Qm_hi,q,s [16x128]

_CACHE = {}


def _build_program(loop_iters: int = 1, pe_only: bool = False,
                   no_dma: bool = False):
    """pe_only=True builds a timing-diagnostic variant with the drain
    copies and output DMAs removed from the loop (output stays zero);
    no_dma=True keeps the drains but skips the output DMAs."""
    import contextlib
    import concourse.bacc as bacc
    import concourse.mybir as mybir
    from concourse.tile import TileContext

    nc = bacc.Bacc("TRN2", target_bir_lowering=False, debug=False,
                   num_devices=NCORES)
    bf16 = mybir.dt.bfloat16
    xin = {}
    for name in ("xm0", "xm1", "xms"):
        xin[name] = nc.dram_tensor(name, [256, ROWS], bf16,
                                   kind="ExternalInput").ap()
    for name in ("xpm0", "xpm1", "xppp", "xppm"):
        xin[name] = nc.dram_tensor(name, [128, ROWS], bf16,
                                   kind="ExternalInput").ap()
    wk1 = nc.dram_tensor("wk1", [128, 2 * 3 * D * 256], bf16,
                         kind="ExternalInput").ap()
    wk2 = nc.dram_tensor("wk2", [128, 3 * D * 128], bf16,
                         kind="ExternalInput").ap()
    wk3 = nc.dram_tensor("wk3", [128, 2 * D * 128], bf16,
                         kind="ExternalInput").ap()
    out = nc.dram_tensor("out", [ROWS, DEV_COLS], bf16,
                         kind="ExternalOutput").ap()

    with TileContext(nc) as tc:
        with (
            tc.tile_pool(name="xp", bufs=1) as xpool,
            tc.tile_pool(name="wp", bufs=1) as wpool,
            tc.tile_pool(name="ps", bufs=2, space="PSUM") as pspool,
            tc.tile_pool(name="ob", bufs=8) as opool,
        ):
            # resident transposed folded activations
            xt = {}
            for name in ("xm0", "xm1", "xms"):
                tiles = []
                for k in range(2):
                    t = xpool.tile([128, ROWS], bf16, tag=f"{name}_{k}")
                    nc.sync.dma_start(
                        out=t[:], in_=xin[name][k * 128:(k + 1) * 128, :])
                    tiles.append(t)
                xt[name] = tiles
            for name in ("xpm0", "xpm1", "xppp", "xppm"):
                t = xpool.tile([128, ROWS], bf16, tag=name)
                nc.sync.dma_start(out=t[:], in_=xin[name][:, :])
                xt[name] = t

            # resident weights (decoder-contiguous direct row blocks)
            wk1t = wpool.tile([128, 2 * 3 * D * 256], bf16, tag="wk1")
            nc.sync.dma_start(out=wk1t[:], in_=wk1[:, :])
            wk2t = wpool.tile([128, 3 * D * 128], bf16, tag="wk2")
            nc.sync.dma_start(out=wk2t[:], in_=wk2[:, :])
            wk3t = wpool.tile([128, 2 * D * 128], bf16, tag="wk3")
            nc.sync.dma_start(out=wk3t[:], in_=wk3[:, :])

            loop_cm = (tc.For_i(0, loop_iters, 1,
                                hint_engines=(mybir.EngineType.PE,
                                              mybir.EngineType.SP,
                                              mybir.EngineType.DVE,
                                              mybir.EngineType.Activation))
                       if loop_iters > 1 else contextlib.nullcontext())
            with loop_cm:
              for m in range(16):
                ms = slice(m * 128, (m + 1) * 128)
                # phase drains accumulate into stage tiles flushed every
                # FLUSH_COLS -- finer DMA pipelining, same total SBUF
                st = {"tile": None, "fill": 0, "col": 0, "flip": 0}

                def drain_dma(ps, width):
                    """One full-width drain copy per phase, alternating
                    DVE/ACT (fewer ops -> less per-op sync overhead)."""
                    if pe_only:
                        return
                    if st["tile"] is None:
                        st["tile"] = opool.tile([128, FLUSH_COLS], bf16,
                                                name="stage")
                        st["fill"] = 0
                    stage, f0 = st["tile"], st["fill"]
                    engs = (nc.vector.tensor_copy, nc.scalar.copy)
                    f = st["flip"]
                    st["flip"] ^= 1
                    engs[f](out=stage[:, f0:f0 + width], in_=ps[:, 0:width])
                    st["fill"] += width
                    if st["fill"] >= FLUSH_COLS:
                        if not no_dma:
                            nc.sync.dma_start(
                                out=out[ms,
                                        st["col"]:st["col"] + st["fill"]],
                                in_=stage[:, 0:st["fill"]])
                        st["col"] += st["fill"]
                        st["tile"] = None

                # m-leaves (K=256): one phase per Karatsuba matrix per
                # 8-decoder half; each LDWEIGHTS feeds 4 N=512 matmuls.
                xmk = (xt["xm0"], xt["xm1"], xt["xms"])
                for mat in range(3):
                    for h in range(2):
                        ps = pspool.tile([128, 2048], mybir.dt.float32,
                                         name="ps")
                        for k in range(2):
                            lhsT = xmk[mat][k][:, ms]
                            for p in range(4):
                                w0 = ((k * 3 + mat) * D + 8 * h
                                      + 2 * p) * 256
                                nc.tensor.matmul(
                                    ps[:, p * 512:(p + 1) * 512], lhsT,
                                    wk1t[:, w0:w0 + 512],
                                    start=(k == 0), stop=(k == 1))
                        drain_dma(ps, 2048)

                # neg256 direct block form, PSUM-accumulated (K=128 each):
                #   Qm_lo = xpm0 @ P2 + xpm1 @ R2
                #   Qm_hi = xpm1 @ P2 - xpm0 @ R2
                # (wk2 stores [P2 | R2 | -R2] 16-decoder blocks)
                for ops in (((xt["xpm0"], 0), (xt["xpm1"], 1)),
                            ((xt["xpm1"], 0), (xt["xpm0"], 2))):
                    ps = pspool.tile([128, 2048], mybir.dt.float32,
                                     name="ps")
                    for step, (lx, mat) in enumerate(ops):
                        lhsT = lx[:, ms]
                        for p in range(4):
                            w0 = (mat * D + 4 * p) * 128
                            nc.tensor.matmul(
                                ps[:, p * 512:(p + 1) * 512], lhsT,
                                wk2t[:, w0:w0 + 512],
                                start=(step == 0), stop=(step == 1))
                    drain_dma(ps, 2048)

                # cyc128/neg128 leaves (K=128): one phase per leaf spanning
                # all 16 decoders; one LDWEIGHTS feeds 4 N=512 matmuls.
                for lx, mat in ((xt["xppp"], 0), (xt["xppm"], 1)):
                    ps = pspool.tile([128, 2048], mybir.dt.float32,
                                     name="ps")
                    lhsT = lx[:, ms]
                    for p in range(4):
                        w0 = (mat * D + 4 * p) * 128
                        nc.tensor.matmul(
                            ps[:, p * 512:(p + 1) * 512], lhsT,
                            wk3t[:, w0:w0 + 512], start=True, stop=True)
                    drain_dma(ps, 2048)
    _finalize_with_dedup(nc)
    return nc


def _dedup_ldweights(nc):
    """Drop redundant InstLdweights from the PE stream.

    bacc emits every matmul as an (InstLdweights, InstMatmult) pair; the
    matmult is non-self-loading, so the PE weight register persists across
    matmuls.  Consecutive pairs with an identical stationary AP reload the
    same weights (~107ns each on HW).  Drop an InstLdweights when its
    signature matches the previous one on the PE stream AND it carries no
    waits/updates.  Conservatively resets tracking at block boundaries and
    on any other PE instruction.
    """
    import concourse.mybir as mybir

    InstLdweights = mybir.InstLdweights
    InstMatmult = mybir.InstMatmult
    n_drop = 0
    for fn in nc.m.functions:
        for blk in fn.blocks:
            keep = []
            last_sig = None
            for inst in blk.instructions:
                if isinstance(inst, InstLdweights):
                    pap = inst.ins[0]
                    sig = (pap.memref, pap.offset, str(pap.ap),
                           str(pap.dtype),
                           str(getattr(inst, "perf_mode", None)),
                           str(getattr(inst, "is_transpose", None)),
                           str(getattr(inst, "tile_position", None)))
                    si = inst.sync_info
                    bare = si is None or (len(si.on_wait) == 0
                                          and len(si.on_update) == 0)
                    if sig == last_sig and bare:
                        n_drop += 1
                        continue
                    last_sig = sig
                elif getattr(inst, "engine", None) == mybir.EngineType.PE:
                    if isinstance(inst, InstMatmult):
                        if getattr(inst, "is_transpose", None):
                            last_sig = None
                    else:
                        last_sig = None
                keep.append(inst)
            if n_drop:
                try:
                    blk.instructions = keep
                except Exception:
                    insts = blk.instructions
                    while len(insts):
                        insts.pop()
                    for i in keep:
                        insts.append(i)
    return n_drop


def _finalize_with_dedup(nc):
    orig_mv = nc.move_matmul_waits_to_ldweights

    def _mv():
        orig_mv()
        _dedup_ldweights(nc)

    nc.move_matmul_waits_to_ldweights = _mv
    nc.finalize()


def _get_program(loop_iters: int = 1):
    key = f"nc{loop_iters}"
    if key not in _CACHE:
        _CACHE[key] = _build_program(loop_iters)
    return _CACHE[key]


def _skew(v):
    """Skew-circulant (negacyclic) matrix W[k,j] = v[k-j], -v[k-j+M] below
    the diagonal."""
    M = len(v)
    k = np.arange(M)[:, None]
    j = np.arange(M)[None, :]
    r = k - j
    return np.where(r >= 0, v[r % M], -v[r % M])


def _circ(v):
    M = len(v)
    k = np.arange(M)[:, None]
    j = np.arange(M)[None, :]
    return v[(k - j) % M]


def _build_weights(decoders: np.ndarray):
    """Decoder-contiguous direct row-block weight buffers (bf16).

    wk1[:, ((k*3+mat)*D + d)*256 :][:256]: k-th 128-row block of the 256x256
    Karatsuba matrix mat in {P, R, P-R} of skew512(h_d)/2.
    wk2[:, (mat*D + d)*128 :][:128]: {P2, R2, -R2} blocks of
    skew256(gm_d)/4.   wk3: {circ128(gpp_d)/8, skew128(gpm_d)/8}.
    """
    dec = np.asarray(decoders, np.float32)
    wk1 = np.empty((128, 2 * 3 * D * 256), np.float32)
    wk2 = np.empty((128, 3 * D * 128), np.float32)
    wk3 = np.empty((128, 2 * D * 128), np.float32)
    for d in range(D):
        v = dec[d]
        g = v[:512] + v[512:]
        h = (v[:512] - v[512:]) / 2
        gm = (g[:256] - g[256:]) / 4
        gp = g[:256] + g[256:]
        gpp = (gp[:128] + gp[128:]) / 8
        gpm = (gp[:128] - gp[128:]) / 8

        S = _skew(h)                      # 512x512
        mats1 = (S[:256, :256], S[256:, :256],
                 S[:256, :256] - S[256:, :256])     # P, R, P-R
        for mat, W in enumerate(mats1):
            for k in range(2):
                c0 = ((k * 3 + mat) * D + d) * 256
                wk1[:, c0:c0 + 256] = W[k * 128:(k + 1) * 128, :]
        S2 = _skew(gm)                    # 256x256
        mats2 = (S2[:128, :128], S2[128:, :128],
                 -S2[128:, :128])                   # P2, R2, -R2
        for mat, W in enumerate(mats2):
            c0 = (mat * D + d) * 128
            wk2[:, c0:c0 + 128] = W
        for which, W in enumerate((_circ(gpp), _skew(gpm))):
            c0 = (which * D + d) * 128
            wk3[:, c0:c0 + 128] = W
    b = ml_dtypes.bfloat16
    return wk1.astype(b), wk2.astype(b), wk3.astype(b)


def _build_in_maps(problemhrr, lemmahrr, decoders):
    b = ml_dtypes.bfloat16
    wk1, wk2, wk3 = _build_weights(decoders)

    def t(a):
        return np.ascontiguousarray(a.T).astype(b)

    in_maps = []
    for c in range(NCORES):
        p = problemhrr[c * BPC:(c + 1) * BPC]
        l = lemmahrr[c * BPC:(c + 1) * BPC]
        X = np.concatenate([p, l], axis=0)          # [2048, 1024] f32
        xm = X[:, :512] - X[:, 512:]
        xp = X[:, :512] + X[:, 512:]
        xpp = xp[:, :256] + xp[:, 256:]
        xpm = xp[:, :256] - xp[:, 256:]
        xm0, xm1 = xm[:, :256], xm[:, 256:]
        in_maps.append({
            "xm0": t(xm0), "xm1": t(xm1), "xms": t(xm0 + xm1),
            "xpm0": t(xpm[:, :128]), "xpm1": t(xpm[:, 128:]),
            "xppp": t(xpp[:, :128] + xpp[:, 128:]),
            "xppm": t(xpp[:, :128] - xpp[:, 128:]),
            "wk1": wk1, "wk2": wk2, "wk3": wk3,
        })
    return in_maps


def _bf16_to_f32(a: np.ndarray) -> np.ndarray:
    return (a.view(np.uint16).astype(np.uint32) << 16).view(np.float32)


def _recombine(dev_out: np.ndarray) -> np.ndarray:
    """[ROWS, DEV_COLS] bf16 phase-major partials -> [ROWS, D*1024] f32."""
    arr = _bf16_to_f32(dev_out)
    m1 = arr[:, 0:4096].reshape(ROWS, D, 256)
    m2 = arr[:, 4096:8192].reshape(ROWS, D, 256)
    m3 = arr[:, 8192:12288].reshape(ROWS, D, 256)
    qm_lo = arr[:, 12288:14336].reshape(ROWS, D, 128)
    qm_hi = arr[:, 14336:16384].reshape(ROWS, D, 128)
    q = arr[:, 16384:18432].reshape(ROWS, D, 128)
    s = arr[:, 18432:20480].reshape(ROWS, D, 128)

    y = np.empty((ROWS, D, 1024), np.float32)
    t = y[:, :, 0:512]
    Qp_lo = y[:, :, 0:128]              # scratch inside t
    np.add(q, s, out=Qp_lo)
    Qp_hi = y[:, :, 128:256]
    np.subtract(q, s, out=Qp_hi)
    Qp = y[:, :, 0:256]
    Qm = np.concatenate([qm_lo, qm_hi], axis=-1)            # [.,.,256]
    np.subtract(Qp, Qm, out=y[:, :, 256:512])
    np.add(Qp, Qm, out=Qp)              # t = [Qp+Qm, Qp-Qm]
    Pm = np.concatenate([m1 + m2, m3 - m1 + m2], axis=-1)   # [.,.,512]
    np.subtract(t, Pm, out=y[:, :, 512:1024])
    np.add(t, Pm, out=t)
    return y.reshape(ROWS, D * 1024)


def kernel(problemhrr: np.ndarray, lemmahrr: np.ndarray,
           decoders: np.ndarray) -> np.ndarray:
    from concourse.bass_utils import run_bass_kernel_spmd

    problemhrr = np.asarray(problemhrr, dtype=np.float32)
    lemmahrr = np.asarray(lemmahrr, dtype=np.float32)
    decoders = np.asarray(decoders, dtype=np.float32)

    nc = _get_program()
    in_maps = _build_in_maps(problemhrr, lemmahrr, decoders)
    res = run_bass_kernel_spmd(nc, in_maps, list(range(NCORES)))

    full = np.empty((B, OUT_COLS), np.float32)
    full[:, :HRR] = problemhrr
    full[:, HRR:2 * HRR] = lemmahrr
    for c in range(NCORES):
        y = _recombine(res.results[c]["out"])
        rows = slice(c * BPC, (c + 1) * BPC)
        full[rows, 2 * HRR:2 * HRR + DN] = y[:BPC]
        full[rows, 2 * HRR + DN:] = y[BPC:]
    return full


# revision 45
# speedup vs baseline: 1.0083x; 1.0083x over previous
"""Trainium2 kernel for nn_Decoder_featurizer: HRR decoder featurization.

reference: out = concat([p, l, assoc(dec_d, p)..., assoc(dec_d, l)...], -1)
where assoc(d, x)[j] = sum_t d[t] * x[(j+t) % N]  (circular correlation).

Circular correlation is a circulant matmul y = x @ C_d with
C_d[k, j] = dec[d, (k-j) % N].  v5 cuts the PE work 3.5x below the dense
circulant by composing two matrix identities, recursively:

 (1) CRT split of a cyclic ring:  x^2M-1 = (x^M-1)(x^M+1) turns a
     circulant-2M matmul into a circulant-M plus a negacyclic-M matmul on
     folded inputs (2 half-size mults instead of 4 quarters).
 (2) Karatsuba on the negacyclic ring: the skew-circulant has block form
     [[P, -R], [R, P]] (complex-multiplication structure), so it costs 3
     half-size Toeplitz matmuls (m1=x0@P, m2=x1@R, m3=(x0+x1)@(P-R)) with
     the butterfly  y = [m1+m2, m3-m1+m2].

Applied to N=1024:  cyc1024 -> cyc512 + neg512;  cyc512 -> cyc256 + neg256;
cyc256 -> cyc128 + neg128;  neg512 -> 3 x Toep256 (Karatsuba).  Leaves per
decoder:
  m1,m2,m3     : 3 matmuls [B,256]@[256,256]   (neg512 Karatsuba)
  Qm_lo,Qm_hi  : 4 matmuls [B,128]@[128,128]   (neg256 direct block form,
                 pairs PSUM-accumulated: Qm_lo = x0@P2 + x1@R2,
                 Qm_hi = x1@P2 - x0@R2 -- trades 1 extra small matmul for
                 shipping 256 instead of 384 partial cols; the drain/DMA
                 path, not the PE, is the binding budget)
  q            : [B,128]@circ128               (cyc128)
  s            : [B,128]@skew128               (neg128)
= 245760 MACs/row/decoder vs 1048576 dense.  All folded x operands are
host-precomputed (shipped transposed, bf16); the remaining butterfly
recombines run on the host in f32.  The passthrough columns never touch
the device.

Every device matmul has N=512: weights are stored decoder-contiguous so one
moving operand covers 2 decoders (256-wide leaves) or 4 decoders (128-wide
leaves) -- small-N matmuls measurably pay a large per-MM floor (v4 post-
mortem).  v7 organizes the work as single-leaf PSUM phases spanning 8
decoders (256-wide leaves, 4 pairs) or all 16 (128-wide leaves), so each
stationary LDWEIGHTS feeds 4 N=512 matmuls.  Per m-tile: 10 phases of 4
PSUM banks rotating through 2 pool buffers; DVE and ScalarE split each
phase's psum->SBUF bf16 drain so banks free within the next phase's PE
window; all drains land in one stage tile and a single 5.2 MB DMA per
m-tile writes out at ~97%% DMA efficiency.  Device output is raw partials
in bf16, phase-major.  Redundant LDWEIGHTS are deduped at the BIR level.
"""

import numpy as np
import ml_dtypes

HRR = 1024
D = 16
B = 8192
NCORES = 8
BPC = B // NCORES            # batch rows per core
ROWS = 2 * BPC               # rows per core (problem + lemma stacked)
DN = D * HRR                 # 16384 assoc features per input
OUT_COLS = 2 * HRR + 2 * DN  # 34816
DEV_COLS = 20480             # device partial columns per row
FLUSH_COLS = 10240           # stage-tile flush granularity (2.6 MB DMAs;
                             # 1 MB quarter-m flushes measured slower)
# phase-major column layout: m1,m2,m3 [16 dec x 256], Qm_lo,Qm_hi,q,s [16x128]

_CACHE = {}


def _build_program(loop_iters: int = 1, pe_only: bool = False,
                   no_dma: bool = False):
    """pe_only=True builds a timing-diagnostic variant with the drain
    copies and output DMAs removed from the loop (output stays zero);
    no_dma=True keeps the drains but skips the output DMAs."""
    import contextlib
    import concourse.bacc as bacc
    import concourse.mybir as mybir
    from concourse.tile import TileContext

    nc = bacc.Bacc("TRN2", target_bir_lowering=False, debug=False,
                   num_devices=NCORES)
    bf16 = mybir.dt.bfloat16
    xin = {}
    for name in ("xm0", "xm1", "xms"):
        xin[name] = nc.dram_tensor(name, [256, ROWS], bf16,
                                   kind="ExternalInput").ap()
    for name in ("xpm0", "xpm1", "xppp", "xppm"):
        xin[name] = nc.dram_tensor(name, [128, ROWS], bf16,
                                   kind="ExternalInput").ap()
    wk1 = nc.dram_tensor("wk1", [128, 2 * 3 * D * 256], bf16,
                         kind="ExternalInput").ap()
    wk2 = nc.dram_tensor("wk2", [128, 3 * D * 128], bf16,
                         kind="ExternalInput").ap()
    wk3 = nc.dram_tensor("wk3", [128, 2 * D * 128], bf16,
                         kind="ExternalInput").ap()
    out = nc.dram_tensor("out", [ROWS, DEV_COLS], bf16,
                         kind="ExternalOutput").ap()

    with TileContext(nc) as tc:
        with (
            tc.tile_pool(name="xp", bufs=1) as xpool,
            tc.tile_pool(name="wp", bufs=1) as wpool,
            tc.tile_pool(name="ps", bufs=2, space="PSUM") as pspool,
            tc.tile_pool(name="ob", bufs=4) as opool,
        ):
            # resident transposed folded activations
            xt = {}
            for name in ("xm0", "xm1", "xms"):
                tiles = []
                for k in range(2):
                    t = xpool.tile([128, ROWS], bf16, tag=f"{name}_{k}")
                    nc.sync.dma_start(
                        out=t[:], in_=xin[name][k * 128:(k + 1) * 128, :])
                    tiles.append(t)
                xt[name] = tiles
            for name in ("xpm0", "xpm1", "xppp", "xppm"):
                t = xpool.tile([128, ROWS], bf16, tag=name)
                nc.sync.dma_start(out=t[:], in_=xin[name][:, :])
                xt[name] = t

            # resident weights (decoder-contiguous direct row blocks)
            wk1t = wpool.tile([128, 2 * 3 * D * 256], bf16, tag="wk1")
            nc.sync.dma_start(out=wk1t[:], in_=wk1[:, :])
            wk2t = wpool.tile([128, 3 * D * 128], bf16, tag="wk2")
            nc.sync.dma_start(out=wk2t[:], in_=wk2[:, :])
            wk3t = wpool.tile([128, 2 * D * 128], bf16, tag="wk3")
            nc.sync.dma_start(out=wk3t[:], in_=wk3[:, :])

            loop_cm = (tc.For_i(0, loop_iters, 1,
                                hint_engines=(mybir.EngineType.PE,
                                              mybir.EngineType.SP,
                                              mybir.EngineType.DVE,
                                              mybir.EngineType.Activation))
                       if loop_iters > 1 else contextlib.nullcontext())
            with loop_cm:
              for m in range(16):
                ms = slice(m * 128, (m + 1) * 128)
                # phase drains accumulate into stage tiles flushed every
                # FLUSH_COLS -- finer DMA pipelining, same total SBUF
                st = {"tile": None, "fill": 0, "col": 0, "flip": 0}

                def drain_dma(ps, width):
                    """One full-width drain copy per phase, alternating
                    DVE/ACT (fewer ops -> less per-op sync overhead)."""
                    if pe_only:
                        return
                    if st["tile"] is None:
                        st["tile"] = opool.tile([128, FLUSH_COLS], bf16,
                                                name="stage")
                        st["fill"] = 0
                    stage, f0 = st["tile"], st["fill"]
                    engs = (nc.vector.tensor_copy, nc.scalar.copy)
                    f = st["flip"]
                    st["flip"] ^= 1
                    engs[f](out=stage[:, f0:f0 + width], in_=ps[:, 0:width])
                    st["fill"] += width
                    if st["fill"] >= FLUSH_COLS:
                        if not no_dma:
                            nc.sync.dma_start(
                                out=out[ms,
                                        st["col"]:st["col"] + st["fill"]],
                                in_=stage[:, 0:st["fill"]])
                        st["col"] += st["fill"]
                        st["tile"] = None

                # m-leaves (K=256): one phase per Karatsuba matrix per
                # 8-decoder half; each LDWEIGHTS feeds 4 N=512 matmuls.
                xmk = (xt["xm0"], xt["xm1"], xt["xms"])
                for mat in range(3):
                    for h in range(2):
                        ps = pspool.tile([128, 2048], mybir.dt.float32,
                                         name="ps")
                        for k in range(2):
                            lhsT = xmk[mat][k][:, ms]
                            for p in range(4):
                                w0 = ((k * 3 + mat) * D + 8 * h
                                      + 2 * p) * 256
                                nc.tensor.matmul(
                                    ps[:, p * 512:(p + 1) * 512], lhsT,
                                    wk1t[:, w0:w0 + 512],
                                    start=(k == 0), stop=(k == 1))
                        drain_dma(ps, 2048)

                # neg256 direct block form, PSUM-accumulated (K=128 each):
                #   Qm_lo = xpm0 @ P2 + xpm1 @ R2
                #   Qm_hi = xpm1 @ P2 - xpm0 @ R2
                # (wk2 stores [P2 | R2 | -R2] 16-decoder blocks)
                for ops in (((xt["xpm0"], 0), (xt["xpm1"], 1)),
                            ((xt["xpm1"], 0), (xt["xpm0"], 2))):
                    ps = pspool.tile([128, 2048], mybir.dt.float32,
                                     name="ps")
                    for step, (lx, mat) in enumerate(ops):
                        lhsT = lx[:, ms]
                        for p in range(4):
                            w0 = (mat * D + 4 * p) * 128
                            nc.tensor.matmul(
                                ps[:, p * 512:(p + 1) * 512], lhsT,
                                wk2t[:, w0:w0 + 512],
                                start=(step == 0), stop=(step == 1))
                    drain_dma(ps, 2048)

                # cyc128/neg128 leaves (K=128): one phase per leaf spanning
                # all 16 decoders; one LDWEIGHTS feeds 4 N=512 matmuls.
                for lx, mat in ((xt["xppp"], 0), (xt["xppm"], 1)):
                    ps = pspool.tile([128, 2048], mybir.dt.float32,
                                     name="ps")
                    lhsT = lx[:, ms]
                    for p in range(4):
                        w0 = (mat * D + 4 * p) * 128
                        nc.tensor.matmul(
                            ps[:, p * 512:(p + 1) * 512], lhsT,
                            wk3t[:, w0:w0 + 512], start=True, stop=True)
                    drain_dma(ps, 2048)
    _finalize_with_dedup(nc)
    return nc


def _dedup_ldweights(nc):
    """Drop redundant InstLdweights from the PE stream.

    bacc emits every matmul as an (InstLdweights, InstMatmult) pair; the
    matmult is non-self-loading, so the PE weight register persists across
    matmuls.  Consecutive pairs with an identical stationary AP reload the
    same weights (~107ns each on HW).  Drop an InstLdweights when its
    signature matches the previous one on the PE stream AND it carries no
    waits/updates.  Conservatively resets tracking at block boundaries and
    on any other PE instruction.
    """
    import concourse.mybir as mybir

    InstLdweights = mybir.InstLdweights
    InstMatmult = mybir.InstMatmult
    n_drop = 0
    for fn in nc.m.functions:
        for blk in fn.blocks:
            keep = []
            last_sig = None
            for inst in blk.instructions:
                if isinstance(inst, InstLdweights):
                    pap = inst.ins[0]
                    sig = (pap.memref, pap.offset, str(pap.ap),
                           str(pap.dtype),
                           str(getattr(inst, "perf_mode", None)),
                           str(getattr(inst, "is_transpose", None)),
                           str(getattr(inst, "tile_position", None)))
                    si = inst.sync_info
                    bare = si is None or (len(si.on_wait) == 0
                                          and len(si.on_update) == 0)
                    if sig == last_sig and bare:
                        n_drop += 1
                        continue
                    last_sig = sig
                elif getattr(inst, "engine", None) == mybir.EngineType.PE:
                    if isinstance(inst, InstMatmult):
                        if getattr(inst, "is_transpose", None):
                            last_sig = None
                    else:
                        last_sig = None
                keep.append(inst)
            if n_drop:
                try:
                    blk.instructions = keep
                except Exception:
                    insts = blk.instructions
                    while len(insts):
                        insts.pop()
                    for i in keep:
                        insts.append(i)
    return n_drop


def _finalize_with_dedup(nc):
    orig_mv = nc.move_matmul_waits_to_ldweights

    def _mv():
        orig_mv()
        _dedup_ldweights(nc)

    nc.move_matmul_waits_to_ldweights = _mv
    nc.finalize()


def _get_program(loop_iters: int = 1):
    key = f"nc{loop_iters}"
    if key not in _CACHE:
        _CACHE[key] = _build_program(loop_iters)
    return _CACHE[key]


def _skew(v):
    """Skew-circulant (negacyclic) matrix W[k,j] = v[k-j], -v[k-j+M] below
    the diagonal."""
    M = len(v)
    k = np.arange(M)[:, None]
    j = np.arange(M)[None, :]
    r = k - j
    return np.where(r >= 0, v[r % M], -v[r % M])


def _circ(v):
    M = len(v)
    k = np.arange(M)[:, None]
    j = np.arange(M)[None, :]
    return v[(k - j) % M]


def _build_weights(decoders: np.ndarray):
    """Decoder-contiguous direct row-block weight buffers (bf16).

    wk1[:, ((k*3+mat)*D + d)*256 :][:256]: k-th 128-row block of the 256x256
    Karatsuba matrix mat in {P, R, P-R} of skew512(h_d)/2.
    wk2[:, (mat*D + d)*128 :][:128]: {P2, R2, -R2} blocks of
    skew256(gm_d)/4.   wk3: {circ128(gpp_d)/8, skew128(gpm_d)/8}.
    """
    dec = np.asarray(decoders, np.float32)
    wk1 = np.empty((128, 2 * 3 * D * 256), np.float32)
    wk2 = np.empty((128, 3 * D * 128), np.float32)
    wk3 = np.empty((128, 2 * D * 128), np.float32)
    for d in range(D):
        v = dec[d]
        g = v[:512] + v[512:]
        h = (v[:512] - v[512:]) / 2
        gm = (g[:256] - g[256:]) / 4
        gp = g[:256] + g[256:]
        gpp = (gp[:128] + gp[128:]) / 8
        gpm = (gp[:128] - gp[128:]) / 8

        S = _skew(h)                      # 512x512
        mats1 = (S[:256, :256], S[256:, :256],
                 S[:256, :256] - S[256:, :256])     # P, R, P-R
        for mat, W in enumerate(mats1):
            for k in range(2):
                c0 = ((k * 3 + mat) * D + d) * 256
                wk1[:, c0:c0 + 256] = W[k * 128:(k + 1) * 128, :]
        S2 = _skew(gm)                    # 256x256
        mats2 = (S2[:128, :128], S2[128:, :128],
                 -S2[128:, :128])                   # P2, R2, -R2
        for mat, W in enumerate(mats2):
            c0 = (mat * D + d) * 128
            wk2[:, c0:c0 + 128] = W
        for which, W in enumerate((_circ(gpp), _skew(gpm))):
            c0 = (which * D + d) * 128
            wk3[:, c0:c0 + 128] = W
    b = ml_dtypes.bfloat16
    return wk1.astype(b), wk2.astype(b), wk3.astype(b)


def _build_in_maps(problemhrr, lemmahrr, decoders):
    b = ml_dtypes.bfloat16
    wk1, wk2, wk3 = _build_weights(decoders)

    def t(a):
        return np.ascontiguousarray(a.T).astype(b)

    in_maps = []
    for c in range(NCORES):
        p = problemhrr[c * BPC:(c + 1) * BPC]
        l = lemmahrr[c * BPC:(c + 1) * BPC]
        X = np.concatenate([p, l], axis=0)          # [2048, 1024] f32
        xm = X[:, :512] - X[:, 512:]
        xp = X[:, :512] + X[:, 512:]
        xpp = xp[:, :256] + xp[:, 256:]
        xpm = xp[:, :256] - xp[:, 256:]
        xm0, xm1 = xm[:, :256], xm[:, 256:]
        in_maps.append({
            "xm0": t(xm0), "xm1": t(xm1), "xms": t(xm0 + xm1),
            "xpm0": t(xpm[:, :128]), "xpm1": t(xpm[:, 128:]),
            "xppp": t(xpp[:, :128] + xpp[:, 128:]),
            "xppm": t(xpp[:, :128] - xpp[:, 128:]),
            "wk1": wk1, "wk2": wk2, "wk3": wk3,
        })
    return in_maps


def _bf16_to_f32(a: np.ndarray) -> np.ndarray:
    return (a.view(np.uint16).astype(np.uint32) << 16).view(np.float32)


def _recombine(dev_out: np.ndarray) -> np.ndarray:
    """[ROWS, DEV_COLS] bf16 phase-major partials -> [ROWS, D*1024] f32."""
    arr = _bf16_to_f32(dev_out)
    m1 = arr[:, 0:4096].reshape(ROWS, D, 256)
    m2 = arr[:, 4096:8192].reshape(ROWS, D, 256)
    m3 = arr[:, 8192:12288].reshape(ROWS, D, 256)
    qm_lo = arr[:, 12288:14336].reshape(ROWS, D, 128)
    qm_hi = arr[:, 14336:16384].reshape(ROWS, D, 128)
    q = arr[:, 16384:18432].reshape(ROWS, D, 128)
    s = arr[:, 18432:20480].reshape(ROWS, D, 128)

    y = np.empty((ROWS, D, 1024), np.float32)
    t = y[:, :, 0:512]
    Qp_lo = y[:, :, 0:128]              # scratch inside t
    np.add(q, s, out=Qp_lo)
    Qp_hi = y[:, :, 128:256]
    np.subtract(q, s, out=Qp_hi)
    Qp = y[:, :, 0:256]
    Qm = np.concatenate([qm_lo, qm_hi], axis=-1)            # [.,.,256]
    np.subtract(Qp, Qm, out=y[:, :, 256:512])
    np.add(Qp, Qm, out=Qp)              # t = [Qp+Qm, Qp-Qm]
    Pm = np.concatenate([m1 + m2, m3 - m1 + m2], axis=-1)   # [.,.,512]
    np.subtract(t, Pm, out=y[:, :, 512:1024])
    np.add(t, Pm, out=t)
    return y.reshape(ROWS, D * 1024)


def kernel(problemhrr: np.ndarray, lemmahrr: np.ndarray,
           decoders: np.ndarray) -> np.ndarray:
    from concourse.bass_utils import run_bass_kernel_spmd

    problemhrr = np.asarray(problemhrr, dtype=np.float32)
    lemmahrr = np.asarray(lemmahrr, dtype=np.float32)
    decoders = np.asarray(decoders, dtype=np.float32)

    nc = _get_program()
    in_maps = _build_in_maps(problemhrr, lemmahrr, decoders)
    res = run_bass_kernel_spmd(nc, in_maps, list(range(NCORES)))

    full = np.empty((B, OUT_COLS), np.float32)
    full[:, :HRR] = problemhrr
    full[:, HRR:2 * HRR] = lemmahrr
    for c in range(NCORES):
        y = _recombine(res.results[c]["out"])
        rows = slice(c * BPC, (c + 1) * BPC)
        full[rows, 2 * HRR:2 * HRR + DN] = y[:BPC]
        full[rows, 2 * HRR + DN:] = y[BPC:]
    return full
